# revision 24
# baseline (speedup 1.0000x reference)
"""Distributed GQA attention (B=1, T=2048, D=2048, 16 Q heads / 8 KV heads,
head_dim=128, interleaved RoPE, causal) on 8 TRN2 NeuronCores.

Sharding: tensor-parallel over heads. Core c owns Q heads {2c, 2c+1} and KV
head c (exactly the GQA group): 256 columns of Wq, 128+128 columns of Wkv,
256 columns of Wo. After local attention, per-512-q-chunk attention outputs
(transposed [feat, T] layout) are AllGathered (4 chunked collectives that
overlap compute). Each core computes a 256-column shard of the final
projection; the host stitches the 8 column shards (pure relayout).

Perf structure vs the naive version:
- fp16 compute (PE 1 cyc/row, like bf16, but 8x less quantization noise)
- projection superblocks and attention superblocks interleaved
  (P0 P1 A0 P2 A1 P3 A2 A3) so collectives spread across the kernel
- causal-diagonal S/AV/rowsum matmuls run at partial width (no wasted cols)
- causal mask applied additively by a small PE matmul into the S psum
- softmax normalization: rowsum via PE (scaled by 1/16 through V to bound
  fp16 range), broadcast via PE, reciprocal_approx_fast on DVE
- weights and x^T DMA'd in per-j chunks so the first projection matmul
  starts as soon as ~0.3MB has landed; x^T tiles rotate through a 32-slot
  pool (2 superblocks of prefetch)
"""

import numpy as np

import concourse.bass as bass
import concourse.mybir as mybir
from concourse import bacc, tile
from concourse.bass_utils import run_bass_kernel_spmd

F32 = mybir.dt.float32
F16 = mybir.dt.float16
NPF16 = np.float16

P = 128
T = 2048
D = 2048
NC = 8          # cores
HQ = 2          # q heads per core
DH = 128        # head dim
NT = T // P     # 16 k/t blocks
QS = 512        # q super-block width
NQS = T // QS   # 4
ND = D // P     # 16 feature blocks
SCALE = 1.0 / float(np.sqrt(DH))
VSC = 1.0 / 16.0  # V (and rowsum) pre-scale to bound fp16 range


def _rope_tables():
    inv_freq = 1.0 / (10000.0 ** (np.arange(0, DH, 2, dtype=np.float64) / DH))
    ang = np.arange(T, dtype=np.float64)[None, :] * inv_freq[:, None]  # [64, T]
    cos = np.cos(ang)
    sin = np.sin(ang)
    ctab = np.empty((DH, T), np.float32)
    stab = np.empty((DH, T), np.float32)
    ctab[0::2] = cos
    ctab[1::2] = cos
    stab[0::2] = -sin   # row 2i:   out = q[2i]*c - q[2i+1]*s
    stab[1::2] = sin    # row 2i+1: out = q[2i+1]*c + q[2i]*s
    return ctab.astype(NPF16), stab.astype(NPF16)


def _maskneg():
    # maskneg[k, j] = -30000 where q-local j < k (strictly above causal diag
    # in [k, q] layout), else 0. Added into the S psum before exp.
    m = np.zeros((P, P), NPF16)
    k = np.arange(P)[:, None]
    j = np.arange(P)[None, :]
    m[j < k] = -30000.0
    return m


def _perm():
    # permQT = PM @ QT swaps even/odd partner rows
    pm = np.zeros((P, P), NPF16)
    for i in range(0, P, 2):
        pm[i, i + 1] = 1.0
        pm[i + 1, i] = 1.0
    return pm


def build_nc():
    nc = bacc.Bacc(num_devices=NC)

    xt_e = nc.declare_dram_parameter("xt", [D, T], F16, isOutput=False)
    wq_e = nc.declare_dram_parameter("wq", [P, ND * HQ * DH], F16, isOutput=False)
    wk_e = nc.declare_dram_parameter("wk", [P, ND * DH], F16, isOutput=False)
    wv_e = nc.declare_dram_parameter("wv", [P, ND * DH], F16, isOutput=False)
    wo_e = nc.declare_dram_parameter("wo", [P, ND * HQ * DH], F16, isOutput=False)
    bq_e = nc.declare_dram_parameter("bq", [HQ, P], F32, isOutput=False)
    bk_e = nc.declare_dram_parameter("bk", [1, P], F32, isOutput=False)
    bv_e = nc.declare_dram_parameter("bv", [1, P], F32, isOutput=False)
    bo_e = nc.declare_dram_parameter("bo", [HQ, P], F32, isOutput=False)
    ct_e = nc.declare_dram_parameter("costab", [DH, T], F16, isOutput=False)
    st_e = nc.declare_dram_parameter("sintab", [DH, T], F16, isOutput=False)
    mn_e = nc.declare_dram_parameter("maskneg", [P, P], F16, isOutput=False)
    idh_e = nc.declare_dram_parameter("identh", [P, P], F16, isOutput=False)
    pm_e = nc.declare_dram_parameter("perm", [P, P], F16, isOutput=False)
    out_e = nc.declare_dram_parameter("out", [HQ * DH, T], F16, isOutput=True)

    rg = [list(range(NC))]

    with tile.TileContext(nc) as tc:
        # ---------- long-lived pools ----------
        const = tc.alloc_tile_pool(name="const", bufs=1)
        identh = const.tile([P, P], F16)
        nc.sync.dma_start(out=identh[:], in_=idh_e[:])
        permh = const.tile([P, P], F16)
        nc.sync.dma_start(out=permh[:], in_=pm_e[:])
        maskneg = const.tile([P, P], F16)
        nc.sync.dma_start(out=maskneg[:], in_=mn_e[:])
        ones_col = const.tile([P, 1], F16)
        nc.any.memset(ones_col[:], VSC)  # rowsum scaled to match V scaling
        ones_row = const.tile([1, P], F16)
        nc.any.memset(ones_row[:], 1.0)
        bq_t = const.tile([P, HQ], F32)
        nc.sync.dma_start(out=bq_t[:], in_=bq_e.rearrange("h p -> p h"))
        bk_t = const.tile([P, 1], F32)
        nc.sync.dma_start(out=bk_t[:], in_=bk_e.rearrange("h p -> p h"))
        bv_t = const.tile([P, 1], F32)
        nc.sync.dma_start(out=bv_t[:], in_=bv_e.rearrange("h p -> p h"))
        bo_t = const.tile([P, HQ], F32)
        nc.sync.dma_start(out=bo_t[:], in_=bo_e.rearrange("h p -> p h"))
        ctab = const.tile([DH, T], F16)
        nc.sync.dma_start(out=ctab[:], in_=ct_e[:])
        stab = const.tile([DH, T], F16)
        nc.sync.dma_start(out=stab[:], in_=st_e[:])

        wpool = tc.alloc_tile_pool(name="wpool", bufs=1)
        wq_sb = wpool.tile([P, ND * HQ * DH], F16)
        nc.sync.dma_start(out=wq_sb[:], in_=wq_e[:])
        wk_sb = wpool.tile([P, ND * DH], F16)
        nc.sync.dma_start(out=wk_sb[:], in_=wk_e[:])
        wv_sb = wpool.tile([P, ND * DH], F16)
        nc.sync.dma_start(out=wv_sb[:], in_=wv_e[:])
        wo_sb = wpool.tile([P, ND * HQ * DH], F16)

        dram = tc.alloc_tile_pool(name="dram", bufs=1, space="DRAM")
        bar_in = dram.tile([P, 1], F16, name="barin")
        bar_out = dram.tile([NC * P, 1], F16, name="barout",
                            addr_space="Shared")
        # skew barrier: completes when the slowest core has launched, long
        # before AG0 needs the CC engine; nothing consumes bar_out.
        nc.gpsimd.collective_compute(
            "AllGather", mybir.AluOpType.bypass, replica_groups=rg,
            ins=[bar_in.opt()], outs=[bar_out.opt()])
        agin = [dram.tile([HQ * P, QS], F16, name=f"agin{q}")
                for q in range(NQS)]
        agout = [dram.tile([NC * HQ * P, QS], F16, name=f"agout{q}",
                           addr_space="Shared") for q in range(NQS)]

        rope_pool = tc.alloc_tile_pool(name="ropeo", bufs=1)
        q_r = [rope_pool.tile([P, T], F16, name=f"qr{h}") for h in range(HQ)]
        k_r = rope_pool.tile([P, T], F16)

        vnat_pool = tc.alloc_tile_pool(name="vnat", bufs=1)
        v_nat = [vnat_pool.tile([P, DH], F16, name=f"vnat{n}") for n in range(NT)]

        oloc_pool = tc.alloc_tile_pool(name="oloc", bufs=1)
        o_loc = [oloc_pool.tile([P, T], F16, name=f"oloc{h}") for h in range(HQ)]

        ptpool = tc.alloc_tile_pool(name="ptpool", bufs=33)
        norm_pool = tc.alloc_tile_pool(name="normp", bufs=2)

        # ---------- staged x^T DMA: ns=0 chunks first ----
        xT_pool = tc.alloc_tile_pool(name="xTp", bufs=1)
        xTt = [xT_pool.tile([P, T], F16, name=f"xTt{j}") for j in range(ND)]
        xT = [[xTt[j][:, ns * QS:(ns + 1) * QS] for j in range(ND)]
              for ns in range(NQS)]
        for j in range(ND):  # chunk ns=0 first so P(0) can start ASAP
            nc.sync.dma_start(out=xT[0][j],
                              in_=xt_e[j * P:(j + 1) * P, 0:QS])
        nc.sync.dma_start(out=wo_sb[:], in_=wo_e[:])
        for ns in range(1, NQS):
            for j in range(ND):
                sl = slice(ns * QS, (ns + 1) * QS)
                nc.sync.dma_start(out=xT[ns][j],
                                  in_=xt_e[j * P:(j + 1) * P, sl])

        qtmp_pool = tc.alloc_tile_pool(name="qtmp", bufs=2)
        rtmp_pool = tc.alloc_tile_pool(name="rtmp", bufs=2)
        ag_pool = tc.alloc_tile_pool(name="agsb", bufs=32)
        fin_pool = tc.alloc_tile_pool(name="finsb", bufs=2)

        ag_sb = {}

        def fetch_ag(ns):
            for b in range(NC * HQ):
                t = ag_pool.tile([P, QS], F16, name=f"ag{ns}_{b}", tag="ag")
                nc.sync.dma_start(out=t[:], in_=agout[ns][b * P:(b + 1) * P, :])
                ag_sb[(ns, b)] = t

        # ---------- projection superblock ----------
        def proj_block(ns):
            tsl = slice(ns * QS, (ns + 1) * QS)
            with tc.tile_pool(name=f"projp{ns}", bufs=1, space="PSUM") as pj, \
                 tc.tile_pool(name=f"auxp{ns}", bufs=2, space="PSUM") as aux:
                ps_q = [pj.tile([P, QS], F32, tag=f"pq{h}", name=f"psq{ns}{h}")
                        for h in range(HQ)]
                ps_k = pj.tile([P, QS], F32, tag="pk", name=f"psk{ns}")
                ps_v = pj.tile([P, QS], F32, tag="pv", name=f"psv{ns}")
                for j in range(ND):
                    st = (j == 0)
                    sp = (j == ND - 1)
                    for h in range(HQ):
                        nc.tensor.matmul(
                            ps_q[h][:],
                            lhsT=wq_sb[:, j * HQ * DH + h * DH:
                                       j * HQ * DH + h * DH + P],
                            rhs=xT[ns][j], start=st, stop=sp)
                    nc.tensor.matmul(ps_k[:],
                                     lhsT=wk_sb[:, j * DH:j * DH + P],
                                     rhs=xT[ns][j], start=st, stop=sp)
                    nc.tensor.matmul(ps_v[:],
                                     lhsT=wv_sb[:, j * DH:j * DH + P],
                                     rhs=xT[ns][j], start=st, stop=sp)

                # evac + rope for q0, q1, k; evac + transpose for v
                def rope(src_ps, bias, dst):
                    qt = qtmp_pool.tile([P, QS], F16, tag="qt")
                    nc.scalar.activation(
                        out=qt[:], in_=src_ps[:],
                        func=mybir.ActivationFunctionType.Identity, bias=bias)
                    pp = aux.tile([P, QS], F32, tag="aux", name=f"pp{ns}")
                    nc.tensor.matmul(pp[:], lhsT=permh[:], rhs=qt[:],
                                     start=True, stop=True)
                    t1 = rtmp_pool.tile([P, QS], F16, tag="t1")
                    nc.vector.tensor_mul(t1[:], pp[:], stab[:, tsl])
                    t2 = rtmp_pool.tile([P, QS], F16, tag="t2")
                    nc.vector.tensor_mul(t2[:], qt[:], ctab[:, tsl])
                    nc.vector.tensor_add(dst[:, tsl], t1[:], t2[:])

                for h in range(HQ):
                    rope(ps_q[h], bq_t[:, h:h + 1], q_r[h])
                rope(ps_k, bk_t[:, 0:1], k_r)

                vt = qtmp_pool.tile([P, QS], F16, tag="vt", name=f"vt{ns}")
                nc.scalar.activation(
                    out=vt[:], in_=ps_v[:],
                    func=mybir.ActivationFunctionType.Identity, bias=bv_t[:, 0:1])
                for i in range(4):
                    vp = aux.tile([P, P], F16, tag="aux", name=f"vp{ns}_{i}")
                    nc.tensor.transpose(vp[:], vt[:, i * P:(i + 1) * P],
                                        identh[:])
                    nc.scalar.copy(out=v_nat[4 * ns + i][:], in_=vp[:])

        # ---------- attention superblock ----------
        def attn_block(qs, fillers=(), fill_from_pass2=False):
            qsl = slice(qs * QS, (qs + 1) * QS)
            nkb = 4 * (qs + 1)
            fillers = list(fillers)
            fi = 0
            nsteps = nkb if fill_from_pass2 else 2 * nkb
            with tc.tile_pool(name=f"spsum{qs}", bufs=3, space="PSUM") as spsum, \
                 tc.tile_pool(name=f"opsum{qs}", bufs=1, space="PSUM") as opsum, \
                 tc.tile_pool(name=f"rpsum{qs}", bufs=1, space="PSUM") as rpsum:
                o_ps = [opsum.tile([P, QS], F32, tag=f"o{i}", name=f"ops{qs}_{i}")
                        for i in range(HQ)]
                r2_ps = rpsum.tile([P, QS], F32, tag="r", name=f"rps{qs}")
                pts = {}
                # pass 1: S matmuls (+ additive causal mask on the diagonal
                # block); exp pipelines behind on the Scalar engine
                for kb in range(nkb):
                    if not fill_from_pass2:
                        take = (len(fillers) * (kb + 1)) // nsteps
                        while fi < take:
                            fillers[fi]()
                            fi += 1
                    i = kb - 4 * qs
                    w = QS if i < 0 else QS - i * P
                    qoff = qs * QS + (QS - w)
                    for h in range(HQ):
                        s_ps = spsum.tile([P, QS], F32, tag="s",
                                          name=f"sps{qs}_{kb}_{h}")
                        nc.tensor.matmul(s_ps[:, :w],
                                         lhsT=k_r[:, kb * P:(kb + 1) * P],
                                         rhs=q_r[h][:, qoff:qoff + w],
                                         start=True, stop=(i < 0))
                        if i >= 0:
                            nc.tensor.matmul(s_ps[:, 0:P], lhsT=identh[:],
                                             rhs=maskneg[:],
                                             start=False, stop=True,
                                             skip_group_check=True)
                        pt = ptpool.tile([P, QS], F16, tag="pt",
                                         name=f"pt{qs}_{kb}_{h}")
                        nc.scalar.activation(
                            out=pt[:, :w], in_=s_ps[:, :w],
                            func=mybir.ActivationFunctionType.Exp, scale=SCALE)
                        pts[(kb, h)] = (pt, w)
                # pass 2: AV + rowsum matmuls
                for kb in range(nkb):
                    take = (len(fillers) * ((kb + 1) if fill_from_pass2
                                            else (nkb + kb + 1))) // nsteps
                    while fi < take:
                        fillers[fi]()
                        fi += 1
                    for h in range(HQ):
                        pt, w = pts[(kb, h)]
                        co = QS - w
                        nc.tensor.matmul(o_ps[h][:, co:], lhsT=v_nat[kb][:],
                                         rhs=pt[:, :w],
                                         start=(kb == 0), stop=(kb == nkb - 1))
                        nc.tensor.matmul(r2_ps[64 * h:64 * h + 1, co:],
                                         lhsT=ones_col[:], rhs=pt[:, :w],
                                         start=(kb == 0), stop=(kb == nkb - 1),
                                         skip_group_check=True)
                while fi < len(fillers):
                    fillers[fi]()
                    fi += 1
                # normalize each head: r -> bcast -> 1/r (fast approx) -> mul
                for h in range(HQ):
                    rs = norm_pool.tile([1, QS], F16, tag="rs",
                                        name=f"rs{qs}_{h}")
                    nc.scalar.copy(out=rs[:], in_=r2_ps[64 * h:64 * h + 1, :])
                    rb_ps = spsum.tile([P, QS], F32, tag="s",
                                       name=f"rb{qs}_{h}")
                    nc.tensor.matmul(rb_ps[:], lhsT=ones_row[:], rhs=rs[:],
                                     start=True, stop=True)
                    rinv = norm_pool.tile([P, QS], F32, tag="ri",
                                          name=f"ri{qs}_{h}")
                    nc.vector.reciprocal_approx_fast(out=rinv[:], in_=rb_ps[:])
                    nc.vector.tensor_mul(o_loc[h][:, qsl], o_ps[h][:], rinv[:])
                    nc.gpsimd.dma_start(out=agin[qs][h * P:(h + 1) * P, :],
                                        in_=o_loc[h][:, qsl])
            nc.gpsimd.collective_compute(
                "AllGather", mybir.AluOpType.bypass,
                replica_groups=rg,
                ins=[agin[qs].opt()], outs=[agout[qs].opt()])

        def fin_mms(ns, fpsum):
            for m in range(HQ):
                f_ps = None
                def mk(m, b):
                    def emit():
                        nonlocal f_ps
                        if f_ps is None:
                            f_ps = fpsum.tile([P, QS], F32, tag="f",
                                              name=f"fps{ns}_{m}")
                        nc.tensor.matmul(
                            f_ps[:],
                            lhsT=wo_sb[:, b * HQ * DH + m * DH:
                                       b * HQ * DH + m * DH + P],
                            rhs=ag_sb[(ns, b)][:, :],
                            start=(b == 0), stop=(b == NC * HQ - 1))
                    return emit
                for b in range(NC * HQ):
                    yield mk(m, b)
                def evict(m=m):
                    nonlocal f_ps
                    fsb = fin_pool.tile([P, QS], F16, tag="fsb",
                                        name=f"fsb{ns}_{m}")
                    nc.vector.tensor_scalar_add(fsb[:], f_ps[:],
                                                bo_t[:, m:m + 1])
                    nc.sync.dma_start(
                        out=out_e[m * P:(m + 1) * P, ns * QS:(ns + 1) * QS],
                        in_=fsb[:])
                    f_ps = None
                yield evict

        # ---------- emission: P0 P1 A0 P2 A1 P3 A2 A3 + fin tail ----------
        proj_block(0)
        proj_block(1)
        attn_block(0)
        proj_block(2)
        attn_block(1)
        fetch_ag(0)
        proj_block(3)
        fetch_ag(1)

        with tc.tile_pool(name="fpsum", bufs=2, space="PSUM") as fpsum:
            attn_block(2, fillers=list(fin_mms(0, fpsum)) +
                                  list(fin_mms(1, fpsum)))
            fetch_ag(2)
            attn_block(3, fillers=list(fin_mms(2, fpsum)))
            fetch_ag(3)
            for f in fin_mms(3, fpsum):
                f()

        fin_pool.release()
        ag_pool.release()
        rtmp_pool.release()
        qtmp_pool.release()
        xT_pool.release()
        norm_pool.release()
        ptpool.release()
        oloc_pool.release()
        vnat_pool.release()
        rope_pool.release()
        dram.release()
        wpool.release()
        const.release()

    nc.compile()
    return nc


_NC_CACHE = None


def _get_nc():
    global _NC_CACHE
    if _NC_CACHE is None:
        _NC_CACHE = build_nc()
    return _NC_CACHE


def _warr(w):
    # [D, M] -> [P, ND*M]: row p holds feature blocks j at stride M
    m = w.shape[1]
    return np.ascontiguousarray(
        w.reshape(ND, P, m).transpose(1, 0, 2).reshape(P, ND * m)).astype(NPF16)


def _in_maps(x, Wq, bq, Wkv, bkv, Wo, bo):
    x2 = np.asarray(x, np.float32).reshape(T, D)
    xt = np.ascontiguousarray(x2.T).astype(NPF16)
    Wq = np.asarray(Wq, np.float32)
    Wkv = np.asarray(Wkv, np.float32)
    Wo = np.asarray(Wo, np.float32)
    bq = np.asarray(bq, np.float32)
    bkv = np.asarray(bkv, np.float32)
    bo = np.asarray(bo, np.float32)
    ctab, stab = _rope_tables()
    mn = _maskneg()
    pm = _perm()
    identh = np.eye(P, dtype=NPF16)
    NKV = 8
    maps = []
    for c in range(NC):
        qc = slice(HQ * DH * c, HQ * DH * (c + 1))
        kc = slice(DH * c, DH * (c + 1))
        vc = slice(NKV * DH + DH * c, NKV * DH + DH * (c + 1))
        maps.append({
            "xt": xt,
            "wq": _warr(Wq[:, qc]),
            "wk": _warr(Wkv[:, kc]),
            "wv": _warr(Wkv[:, vc] * VSC),
            "wo": _warr(Wo[:, qc]),
            "bq": np.ascontiguousarray(bq[qc]).reshape(HQ, P),
            "bk": np.ascontiguousarray(bkv[kc]).reshape(1, P),
            "bv": np.ascontiguousarray(bkv[vc] * VSC).reshape(1, P),
            "bo": np.ascontiguousarray(bo[qc]).reshape(HQ, P),
            "costab": ctab, "sintab": stab, "maskneg": mn,
            "identh": identh, "perm": pm,
        })
    return maps


def _assemble(results):
    full = np.empty((T, D), np.float32)
    for c in range(NC):
        full[:, HQ * DH * c:HQ * DH * (c + 1)] = \
            results[c]["out"].T.astype(np.float32)
    return full.reshape(1, T, D)


def run(trace=False, tmpdir=None, **inputs):
    nc = _get_nc()
    maps = _in_maps(**inputs)
    res = run_bass_kernel_spmd(nc, maps, core_ids=list(range(NC)), trace=trace,
                               tmpdir=tmpdir)
    return _assemble(res.results), res


def kernel(**inputs):
    out, _ = run(trace=False, **inputs)
    return out
